# revision 13
# baseline (speedup 1.0000x reference)
"""Expert-parallel BaseLayer MoE kernel for 8 TRN2 NeuronCores (slim v5).

Host does everything cheap and precision-critical in exact f32: routing
(argmax affinities), layernorm stats + normalization, the sigmoid gate, the
residual and both biases' final application.  Each core owns one expert and
receives only the normalized tokens routed to it (padded to a common capacity
C), plus that expert's FFN weights.  The device computes ONLY the FLOP-heavy
part: z = relu(xhat @ w1' + b1'), delta = z @ w2, returned as bf16.

Precision/perf split (error budget measured against the 2e-2 gate):
 - matmul-1 entirely in fp8 (e4m3) DoubleRow mode (2 k-tiles per pass, ~1.5x
   bf16 rate).  w1 is pre-scaled by SW1=32 on the host to dodge fp8
   subnormals; 1/SW1 is folded into the ReLU activation's scale.
 - matmul-2: the first NP8 of 32 k2-tiles run in fp8 DoubleRow, the rest in
   bf16, all accumulating into ONE PSUM chain.  The fp8 z is stored as
   z/SZ2 and w2 as w2*SZ2 so fp8 products land at scale 1 (no combine op).

Scheduling notes:
 - weights ride a single SP-ring DMA in exact consumption order (w1 pieces
   then per-i w2 pieces); one ring's DMAs spread over all 16 SDMA engines so
   a single ring reaches full HBM rate and FIFO order prevents w2 starving w1
   (which cost 21us when each stream had its own ring).
 - per-piece weight tiles, because Tile tracks write-deps per tile — one big
   tile would stall the first matmul on the whole 12MB load.
 - DoubleRow token chunks use separate PSUM tiles in matmul-1 because
   start_tensor_calc zero-marks the full 2KB PSUM bank; in matmul-2 the
   full-width bf16 MM runs first (owns the bank zeroing) and the chunked DR
   MMs accumulate with start=False.
"""

import functools
import sys

import numpy as np

for _p in ("/opt/trn_rl_repo", "/opt/pypackages"):
    if _p not in sys.path:
        sys.path.append(_p)

import ml_dtypes  # noqa: E402

import concourse.bass as bass  # noqa: E402
import concourse.mybir as mybir  # noqa: E402
import concourse.tile as tile  # noqa: E402
from concourse import bacc  # noqa: E402
from concourse import bass_utils  # noqa: E402


def _ensure_axon_hooks():
    """bass_utils' trace path imports antenv.axon_hooks, which some agent
    images lack; synthesize it (with the real ctypes NTFF hook when
    available) so tracing degrades gracefully instead of crashing."""
    try:
        import antenv.axon_hooks  # noqa: F401
        return
    except ImportError:
        pass
    import types

    import antenv

    hooks = types.ModuleType("antenv.axon_hooks")
    hooks._hook = None
    hooks.set_axon_ntff_profile_hook = lambda h: setattr(hooks, "_hook", h)
    hooks.get_axon_ntff_profile_hook = lambda: hooks._hook
    sys.modules["antenv.axon_hooks"] = hooks
    antenv.axon_hooks = hooks
    try:
        from trn_agent_boot.trn_boot import _ntff_profile_via_ctypes

        hooks._hook = _ntff_profile_via_ctypes("/opt/axon/libaxon_pjrt.so")
    except Exception:
        pass


_ensure_axon_hooks()

E = 8
D = 1024
F = 4096
EPS = 1e-5
KD = D // 128   # 8 k-tiles over d
KF = F // 128   # 32 k-tiles over f
SW1 = 32.0      # host-side w1 pre-scale (fp8 subnormal avoidance)
SZ2 = 8.0       # z stored as z/SZ2 (fp8), w2 as w2*SZ2 — products at scale 1

NP8 = 12        # k2-tiles of matmul-2 run in fp8 DoubleRow (of KF=32)
W1PC = 16       # w1 DMA pieces (KF/W1PC j's each)

F32 = mybir.dt.float32
BF16 = mybir.dt.bfloat16
F8 = mybir.dt.float8e4
AF = mybir.ActivationFunctionType
DR = mybir.MatmulPerfMode.DoubleRow

NP_BF16 = ml_dtypes.bfloat16
NP_F8 = ml_dtypes.float8_e4m3


def _dr_chunks(c_total):
    """Token chunks for DoubleRow (moving free dim = 2*chunk <= 512)."""
    if c_total <= 256:
        return [c_total]
    h = ((c_total + 1) // 2 + 15) // 16 * 16
    return [h, c_total - h]


def _slices(chunks):
    out, c0 = [], 0
    for cc in chunks:
        out.append(bass.ds(c0, cc))
        c0 += cc
    return out


@functools.lru_cache(maxsize=4)
def _build(c_total, np8):
    nc = bacc.Bacc("TRN2", target_bir_lowering=False, debug=False, num_devices=1)

    nbf = KF - np8  # bf16 k2-tiles in matmul-2
    jpp = KF // W1PC

    xd = nc.declare_dram_parameter("xq", [128, KD, c_total], F8, isOutput=False)
    w1d = nc.declare_dram_parameter("w1q", [128, KF, KD, 128], F8, isOutput=False)
    w28d = nc.declare_dram_parameter("w28", [128, KD, max(np8, 1), 128], F8, isOutput=False)
    w2bd = nc.declare_dram_parameter("w2b", [128, KD, max(nbf, 1), 128], BF16, isOutput=False)
    b1d = nc.declare_dram_parameter("b1c", [128, KF], F32, isOutput=False)
    outd = nc.declare_dram_parameter("out", [128, KD, c_total], BF16, isOutput=True)

    with tile.TileContext(nc) as tc:
        with (
            tc.tile_pool(name="const", bufs=1) as constp,
            tc.tile_pool(name="xp", bufs=1) as xp,
            tc.tile_pool(name="w1p", bufs=1) as w1p,
            tc.tile_pool(name="w2p", bufs=1) as w2p,
            tc.tile_pool(name="zp", bufs=1) as zp,
            tc.tile_pool(name="outp", bufs=3) as outp,
            tc.tile_pool(name="ps_z", bufs=2, space=bass.MemorySpace.PSUM) as psz,
            tc.tile_pool(name="ps_y", bufs=2, space=bass.MemorySpace.PSUM) as psy,
        ):
            # small inputs on the ACT ring (parallel with the SP ring)
            x_sb = xp.tile([128, KD, c_total], F8, tag="x")
            nc.scalar.dma_start(out=x_sb[:], in_=xd[:])
            b1_sb = constp.tile([128, KF], F32, tag="b1")
            nc.scalar.dma_start(out=b1_sb[:], in_=b1d[:])

            # ALL weights ride the single SP ring in exact consumption order
            w1_sbs = []
            for pc in range(W1PC):
                t = w1p.tile([128, jpp, KD, 128], F8, tag=f"w1_{pc}", name=f"w1_{pc}")
                nc.sync.dma_start(out=t[:], in_=w1d[:, jpp * pc : jpp * (pc + 1)])
                w1_sbs.append(t)
            w28_sbs, w2b_sbs = [], []
            for i in range(KD):
                if np8:
                    t8 = w2p.tile([128, np8, 128], F8, tag=f"w28_{i}", name=f"w28_{i}")
                    nc.sync.dma_start(out=t8[:], in_=w28d[:, i])
                    w28_sbs.append(t8)
                if nbf:
                    tb = w2p.tile([128, nbf, 128], BF16, tag=f"w2b_{i}", name=f"w2b_{i}")
                    nc.sync.dma_start(out=tb[:], in_=w2bd[:, i])
                    w2b_sbs.append(tb)

            # z: fp8 (scaled 1/SZ2) for j < np8, bf16 for the rest
            z8_sb = zp.tile([128, max(np8, 1), c_total], F8, tag="z8", name="z8") if np8 else None
            zb_sb = zp.tile([128, max(nbf, 1), c_total], BF16, tag="zb", name="zb") if nbf else None

            chunks = _dr_chunks(c_total)
            sls = _slices(chunks)

            # ---- matmul 1 (all fp8 DR): z = relu((xhat @ w1*SW1)/SW1 + b1) ----
            for j in range(KF):
                w1j = w1_sbs[j // jpp][:, j % jpp]
                pzs = [
                    psz.tile([128, cc], F32, tag=f"pz{ci}", name=f"pz{ci}")
                    for ci, cc in enumerate(chunks)
                ]
                for kp in range(KD // 2):
                    w_ap = w1j[:, 2 * kp : 2 * kp + 2, :]
                    for ci, sl in enumerate(sls):
                        nc.tensor.matmul(
                            pzs[ci][:], w_ap, x_sb[:, 2 * kp : 2 * kp + 2, sl],
                            start=(kp == 0), stop=(kp == KD // 2 - 1),
                            perf_mode=DR,
                        )
                # host pre-divides b1[:, j] by SZ2 for j < np8
                scl = 1.0 / (SW1 * SZ2) if j < np8 else 1.0 / SW1
                dst = z8_sb[:, j] if j < np8 else zb_sb[:, j - np8]
                for ci, sl in enumerate(sls):
                    nc.scalar.activation(
                        dst[:, sl] if len(sls) > 1 else dst[:],
                        pzs[ci][:], AF.Relu,
                        bias=b1_sb[:, j : j + 1], scale=scl,
                    )

            # ---- matmul 2: delta = z8 @ w28 + zb @ w2b (one PSUM chain) ----
            for i in range(KD):
                py = psy.tile([128, c_total], F32, tag="py0")
                if nbf:
                    # full-width bf16 MM first: owns the PSUM bank zeroing
                    nc.tensor.matmul(
                        py[:], w2b_sbs[i][:, 0, :], zb_sb[:, 0, :],
                        start=True, stop=(nbf == 1 and not np8),
                    )
                for kp in range(np8 // 2):
                    w_ap = w28_sbs[i][:, 2 * kp : 2 * kp + 2, :]
                    for ci, sl in enumerate(sls):
                        nc.tensor.matmul(
                            py[:, sl], w_ap, z8_sb[:, 2 * kp : 2 * kp + 2, sl],
                            start=False, stop=False,
                            perf_mode=DR,
                            skip_group_check=True,
                        )
                for k2 in range(1, nbf):
                    nc.tensor.matmul(
                        py[:], w2b_sbs[i][:, k2, :], zb_sb[:, k2, :],
                        start=False, stop=(k2 == nbf - 1),
                        skip_group_check=True,
                    )
                o = outp.tile([128, c_total], BF16, tag="o")
                nc.vector.tensor_copy(o[:], py[:])
                nc.scalar.dma_start(out=outd[:, i, :], in_=o[:])

    nc.compile()
    return nc


def kernel(x, centroids, w1, b1, w2, b2, gamma, beta):
    x = np.ascontiguousarray(np.asarray(x, dtype=np.float32))
    centroids = np.asarray(centroids, dtype=np.float32)
    w1 = np.asarray(w1, dtype=np.float32)
    b1 = np.asarray(b1, dtype=np.float32)
    w2 = np.asarray(w2, dtype=np.float32)
    b2 = np.asarray(b2, dtype=np.float32)
    gamma = np.asarray(gamma, dtype=np.float32)
    beta = np.asarray(beta, dtype=np.float32)

    orig_shape = x.shape
    feats = x.reshape(-1, D)

    # routing + stats + gate — exact f32, same math as the reference
    aff = feats @ centroids.T
    eid = np.argmax(aff, axis=1)
    mu = feats.mean(axis=-1, keepdims=True)
    var = feats.var(axis=-1, keepdims=True)
    xhat = (feats - mu) / np.sqrt(var + EPS)

    idxs = [np.nonzero(eid == e)[0] for e in range(E)]
    counts = [len(ix) for ix in idxs]
    c_total = max(64, ((max(counts) + 31) // 32) * 32)

    nc = _build(c_total, NP8)

    in_maps = []
    for e in range(E):
        n_e = counts[e]
        xt = np.zeros((128, KD, c_total), dtype=np.float32)
        if n_e:
            xt[:, :, :n_e] = xhat[idxs[e]].T.reshape(KD, 128, n_e).transpose(1, 0, 2)
        w1e = gamma[e][:, None] * w1[e]                      # [D, F]
        b1e = b1[e] + beta[e] @ w1[e]                        # [F]
        w1q = np.ascontiguousarray(
            (w1e * SW1).reshape(KD, 128, KF, 128).transpose(1, 2, 0, 3)
        ).astype(NP_F8)                                      # [128,KF,KD,128]
        w2r = w2[e].reshape(KF, 128, KD, 128).transpose(1, 2, 0, 3)  # [128,KD,KF,128]
        w28 = np.ascontiguousarray(w2r[:, :, :NP8] * SZ2).astype(NP_F8)
        w2b = np.ascontiguousarray(w2r[:, :, NP8:]).astype(NP_BF16)
        b1c = np.ascontiguousarray(b1e.reshape(KF, 128).T)   # [128, KF]
        b1c[:, :NP8] /= SZ2
        in_maps.append(
            dict(xq=xt.astype(NP_F8), w1q=w1q, w28=w28, w2b=w2b, b1c=b1c)
        )

    res = bass_utils.run_bass_kernel_spmd(nc, in_maps, core_ids=list(range(E)))
    kernel._last_res = res

    out = feats.copy()
    for e in range(E):
        n_e = counts[e]
        if not n_e:
            continue
        d8 = np.asarray(res.results[e]["out"]).astype(np.float32)  # [128,KD,C]
        delta = d8.transpose(1, 0, 2).reshape(D, c_total)[:, :n_e].T  # [Ce, D]
        al = 1.0 / (1.0 + np.exp(-aff[idxs[e], e]))[:, None]
        out[idxs[e]] = feats[idxs[e]] + al * (delta + b2[e])
    return out.reshape(orig_shape)


# revision 15
# speedup vs baseline: 1.0634x; 1.0634x over previous
"""Expert-parallel BaseLayer MoE kernel for 8 TRN2 NeuronCores (slim v5).

Host does everything cheap and precision-critical in exact f32: routing
(argmax affinities), layernorm stats + normalization, the sigmoid gate, the
residual and both biases' final application.  Each core owns one expert and
receives only the normalized tokens routed to it (padded to a common capacity
C), plus that expert's FFN weights.  The device computes ONLY the FLOP-heavy
part: z = relu(xhat @ w1' + b1'), delta = z @ w2, returned as bf16.

Precision/perf split (error budget measured against the 2e-2 gate):
 - matmul-1 entirely in fp8 (e4m3) DoubleRow mode (2 k-tiles per pass, ~1.5x
   bf16 rate).  w1 is pre-scaled by SW1=32 on the host to dodge fp8
   subnormals; 1/SW1 is folded into the ReLU activation's scale.
 - matmul-2: the first NP8 of 32 k2-tiles run in fp8 DoubleRow, the rest in
   bf16, all accumulating into ONE PSUM chain.  The fp8 z is stored as
   z/SZ2 and w2 as w2*SZ2 so fp8 products land at scale 1 (no combine op).

Scheduling notes:
 - weights ride a single SP-ring DMA in exact consumption order (w1 pieces
   then per-i w2 pieces); one ring's DMAs spread over all 16 SDMA engines so
   a single ring reaches full HBM rate and FIFO order prevents w2 starving w1
   (which cost 21us when each stream had its own ring).
 - per-piece weight tiles, because Tile tracks write-deps per tile — one big
   tile would stall the first matmul on the whole 12MB load.
 - DoubleRow token chunks use separate PSUM tiles in matmul-1 because
   start_tensor_calc zero-marks the full 2KB PSUM bank; in matmul-2 the
   full-width bf16 MM runs first (owns the bank zeroing) and the chunked DR
   MMs accumulate with start=False.
"""

import functools
import sys

import numpy as np

for _p in ("/opt/trn_rl_repo", "/opt/pypackages"):
    if _p not in sys.path:
        sys.path.append(_p)

import ml_dtypes  # noqa: E402

import concourse.bass as bass  # noqa: E402
import concourse.mybir as mybir  # noqa: E402
import concourse.tile as tile  # noqa: E402
from concourse import bacc  # noqa: E402
from concourse import bass_utils  # noqa: E402


def _ensure_axon_hooks():
    """bass_utils' trace path imports antenv.axon_hooks, which some agent
    images lack; synthesize it (with the real ctypes NTFF hook when
    available) so tracing degrades gracefully instead of crashing."""
    try:
        import antenv.axon_hooks  # noqa: F401
        return
    except ImportError:
        pass
    import types

    import antenv

    hooks = types.ModuleType("antenv.axon_hooks")
    hooks._hook = None
    hooks.set_axon_ntff_profile_hook = lambda h: setattr(hooks, "_hook", h)
    hooks.get_axon_ntff_profile_hook = lambda: hooks._hook
    sys.modules["antenv.axon_hooks"] = hooks
    antenv.axon_hooks = hooks
    try:
        from trn_agent_boot.trn_boot import _ntff_profile_via_ctypes

        hooks._hook = _ntff_profile_via_ctypes("/opt/axon/libaxon_pjrt.so")
    except Exception:
        pass


_ensure_axon_hooks()

E = 8
D = 1024
F = 4096
EPS = 1e-5
KD = D // 128   # 8 k-tiles over d
KF = F // 128   # 32 k-tiles over f
SW1 = 32.0      # host-side w1 pre-scale (fp8 subnormal avoidance)
SZ2 = 8.0       # z stored as z/SZ2 (fp8), w2 as w2*SZ2 — products at scale 1

NP8 = 12        # k2-tiles of matmul-2 run in fp8 DoubleRow (of KF=32)
W1PC = 16       # w1 DMA pieces (KF/W1PC j's each)

F32 = mybir.dt.float32
BF16 = mybir.dt.bfloat16
F8 = mybir.dt.float8e4
AF = mybir.ActivationFunctionType
DR = mybir.MatmulPerfMode.DoubleRow

NP_BF16 = ml_dtypes.bfloat16
NP_F8 = ml_dtypes.float8_e4m3


def _dr_chunks(c_total):
    """Token chunks for DoubleRow (moving free dim = 2*chunk <= 512)."""
    if c_total <= 256:
        return [c_total]
    h = ((c_total + 1) // 2 + 15) // 16 * 16
    return [h, c_total - h]


def _slices(chunks):
    out, c0 = [], 0
    for cc in chunks:
        out.append(bass.ds(c0, cc))
        c0 += cc
    return out


@functools.lru_cache(maxsize=4)
def _build(c_total, cw, np8):
    nc = bacc.Bacc("TRN2", target_bir_lowering=False, debug=False, num_devices=E)

    nbf = KF - np8  # bf16 k2-tiles in matmul-2
    jpp = KF // W1PC

    xd = nc.declare_dram_parameter("xq", [128, KD, c_total], F8, isOutput=False)
    w1d = nc.declare_dram_parameter("w1q", [128, KF, KD, 128], F8, isOutput=False)
    w28d = nc.declare_dram_parameter("w28", [128, KD, max(np8, 1), 128], F8, isOutput=False)
    w2bd = nc.declare_dram_parameter("w2b", [128, KD, max(nbf, 1), 128], BF16, isOutput=False)
    b1d = nc.declare_dram_parameter("b1c", [128, KF], F32, isOutput=False)
    outd = nc.declare_dram_parameter("out", [128, KD, c_total], BF16, isOutput=True)

    with tile.TileContext(nc) as tc:
        with (
            tc.tile_pool(name="const", bufs=1) as constp,
            tc.tile_pool(name="xp", bufs=1) as xp,
            tc.tile_pool(name="w1p", bufs=1) as w1p,
            tc.tile_pool(name="w2p", bufs=1) as w2p,
            tc.tile_pool(name="zp", bufs=1) as zp,
            tc.tile_pool(name="outp", bufs=3) as outp,
            tc.tile_pool(name="ps_z", bufs=2, space=bass.MemorySpace.PSUM) as psz,
            tc.tile_pool(name="ps_y", bufs=2, space=bass.MemorySpace.PSUM) as psy,
        ):
            # small inputs on the ACT ring (parallel with the SP ring)
            x_sb = xp.tile([128, KD, c_total], F8, tag="x")
            nc.scalar.dma_start(out=x_sb[:], in_=xd[:])
            b1_sb = constp.tile([128, KF], F32, tag="b1")
            nc.scalar.dma_start(out=b1_sb[:], in_=b1d[:])

            # ALL weights ride the single SP ring in exact consumption order
            w1_sbs = []
            for pc in range(W1PC):
                t = w1p.tile([128, jpp, KD, 128], F8, tag=f"w1_{pc}", name=f"w1_{pc}")
                nc.sync.dma_start(out=t[:], in_=w1d[:, jpp * pc : jpp * (pc + 1)])
                w1_sbs.append(t)
            w28_sbs, w2b_sbs = [], []
            for i in range(KD):
                if np8:
                    t8 = w2p.tile([128, np8, 128], F8, tag=f"w28_{i}", name=f"w28_{i}")
                    nc.sync.dma_start(out=t8[:], in_=w28d[:, i])
                    w28_sbs.append(t8)
                if nbf:
                    tb = w2p.tile([128, nbf, 128], BF16, tag=f"w2b_{i}", name=f"w2b_{i}")
                    nc.sync.dma_start(out=tb[:], in_=w2bd[:, i])
                    w2b_sbs.append(tb)

            # z: fp8 (scaled 1/SZ2) for j < np8, bf16 for the rest
            z8_sb = zp.tile([128, max(np8, 1), c_total], F8, tag="z8", name="z8") if np8 else None
            zb_sb = zp.tile([128, max(nbf, 1), c_total], BF16, tag="zb", name="zb") if nbf else None

            chunks = _dr_chunks(cw)
            sls = _slices(chunks)
            wsl = bass.ds(0, cw)

            # ---- matmul 1 (all fp8 DR): z = relu((xhat @ w1*SW1)/SW1 + b1) ----
            for j in range(KF):
                w1j = w1_sbs[j // jpp][:, j % jpp]
                pzs = [
                    psz.tile([128, cc], F32, tag=f"pz{ci}", name=f"pz{ci}")
                    for ci, cc in enumerate(chunks)
                ]
                for kp in range(KD // 2):
                    w_ap = w1j[:, 2 * kp : 2 * kp + 2, :]
                    for ci, sl in enumerate(sls):
                        nc.tensor.matmul(
                            pzs[ci][:], w_ap, x_sb[:, 2 * kp : 2 * kp + 2, sl],
                            start=(kp == 0), stop=(kp == KD // 2 - 1),
                            perf_mode=DR,
                        )
                # host pre-divides b1[:, j] by SZ2 for j < np8
                scl = 1.0 / (SW1 * SZ2) if j < np8 else 1.0 / SW1
                dst = z8_sb[:, j] if j < np8 else zb_sb[:, j - np8]
                for ci, sl in enumerate(sls):
                    nc.scalar.activation(
                        dst[:, sl] if len(sls) > 1 else dst[:],
                        pzs[ci][:], AF.Relu,
                        bias=b1_sb[:, j : j + 1], scale=scl,
                    )

            # ---- matmul 2: delta = z8 @ w28 + zb @ w2b (one PSUM chain) ----
            for i in range(KD):
                py = psy.tile([128, c_total], F32, tag="py0")
                if nbf:
                    # full-width bf16 MM first: owns the PSUM bank zeroing
                    nc.tensor.matmul(
                        py[:, wsl], w2b_sbs[i][:, 0, :], zb_sb[:, 0, wsl],
                        start=True, stop=(nbf == 1 and not np8),
                    )
                for kp in range(np8 // 2):
                    w_ap = w28_sbs[i][:, 2 * kp : 2 * kp + 2, :]
                    for ci, sl in enumerate(sls):
                        nc.tensor.matmul(
                            py[:, sl], w_ap, z8_sb[:, 2 * kp : 2 * kp + 2, sl],
                            start=False, stop=False,
                            perf_mode=DR,
                            skip_group_check=True,
                        )
                for k2 in range(1, nbf):
                    nc.tensor.matmul(
                        py[:, wsl], w2b_sbs[i][:, k2, :], zb_sb[:, k2, wsl],
                        start=False, stop=(k2 == nbf - 1),
                        skip_group_check=True,
                    )
                o = outp.tile([128, c_total], BF16, tag="o")
                nc.vector.tensor_copy(o[:, wsl], py[:, wsl])
                nc.scalar.dma_start(out=outd[:, i, wsl], in_=o[:, wsl])

    nc.compile()
    return nc


def kernel(x, centroids, w1, b1, w2, b2, gamma, beta):
    x = np.ascontiguousarray(np.asarray(x, dtype=np.float32))
    centroids = np.asarray(centroids, dtype=np.float32)
    w1 = np.asarray(w1, dtype=np.float32)
    b1 = np.asarray(b1, dtype=np.float32)
    w2 = np.asarray(w2, dtype=np.float32)
    b2 = np.asarray(b2, dtype=np.float32)
    gamma = np.asarray(gamma, dtype=np.float32)
    beta = np.asarray(beta, dtype=np.float32)

    orig_shape = x.shape
    feats = x.reshape(-1, D)

    # routing + stats + gate — exact f32, same math as the reference
    aff = feats @ centroids.T
    eid = np.argmax(aff, axis=1)
    mu = feats.mean(axis=-1, keepdims=True)
    var = feats.var(axis=-1, keepdims=True)
    xhat = (feats - mu) / np.sqrt(var + EPS)

    idxs = [np.nonzero(eid == e)[0] for e in range(E)]
    counts = [len(ix) for ix in idxs]
    c_total = max(64, ((max(counts) + 31) // 32) * 32)
    cw = max(64, ((max(counts) + 7) // 8) * 8)

    nc = _build(c_total, cw, NP8)

    in_maps = []
    for e in range(E):
        n_e = counts[e]
        xt = np.zeros((128, KD, c_total), dtype=np.float32)
        if n_e:
            xt[:, :, :n_e] = xhat[idxs[e]].T.reshape(KD, 128, n_e).transpose(1, 0, 2)
        w1e = gamma[e][:, None] * w1[e]                      # [D, F]
        b1e = b1[e] + beta[e] @ w1[e]                        # [F]
        w1q = np.ascontiguousarray(
            (w1e * SW1).reshape(KD, 128, KF, 128).transpose(1, 2, 0, 3)
        ).astype(NP_F8)                                      # [128,KF,KD,128]
        w2r = w2[e].reshape(KF, 128, KD, 128).transpose(1, 2, 0, 3)  # [128,KD,KF,128]
        w28 = np.ascontiguousarray(w2r[:, :, :NP8] * SZ2).astype(NP_F8)
        w2b = np.ascontiguousarray(w2r[:, :, NP8:]).astype(NP_BF16)
        b1c = np.ascontiguousarray(b1e.reshape(KF, 128).T)   # [128, KF]
        b1c[:, :NP8] /= SZ2
        in_maps.append(
            dict(xq=xt.astype(NP_F8), w1q=w1q, w28=w28, w2b=w2b, b1c=b1c)
        )

    res = bass_utils.run_bass_kernel_spmd(nc, in_maps, core_ids=list(range(E)))
    kernel._last_res = res

    out = feats.copy()
    for e in range(E):
        n_e = counts[e]
        if not n_e:
            continue
        d8 = np.asarray(res.results[e]["out"]).astype(np.float32)  # [128,KD,C]
        delta = d8.transpose(1, 0, 2).reshape(D, c_total)[:, :n_e].T  # [Ce, D]
        al = 1.0 / (1.0 + np.exp(-aff[idxs[e], e]))[:, None]
        out[idxs[e]] = feats[idxs[e]] + al * (delta + b2[e])
    return out.reshape(orig_shape)
